# revision 1
# baseline (speedup 1.0000x reference)
"""NuFFT forward (KbNufft-style) Trainium2 Bass kernel.

Strategy:
  - Visibilities only touch |k| <= ~400 of the 2048-point oversampled grid
    (UMAX bound), so each of the 8 cores computes a 105-row x 804-col slab of
    the spectrum via DFT matmuls (apodization folded into the DFT constants):
        slab = Fv_rows . (cube/apod) . Fu_cols^T
  - Visibilities are sharded across cores by their v-row bin, so every
    core's slab fully covers its own visibilities' 6x6 KB footprints.
  - The slab is stored channel-interleaved in DRAM (row = [col][chan][re/im],
    padded to a 256B-multiple row stride); the 6x6 interpolation becomes bulk
    gpsimd.dma_gather calls (256B descriptors; visibilities binned by
    col-offset residue j0%8 so int16 indices address 64-f32-aligned starts
    from an 8*r f32 base offset), then a DVE multiply-reduce against
    host-precomputed 48-tap weight products (6 rows x 8 cols, last 2 zero).
"""
import os
import sys

for _p in ("/opt/trn_rl_repo",):
    if _p not in sys.path and os.path.isdir(_p):
        sys.path.insert(0, _p)

import numpy as np

# ---- problem constants (must match reference.py) ----
NCH = 4
NPIX = 1024
NVIS = 200_000
G = 2048
J = 6
OSF = 2
CELL_ARCSEC = 0.005
DL = CELL_ARCSEC * np.pi / (180.0 * 3600.0)
BETA = float(np.pi * np.sqrt((J / OSF) ** 2 * (OSF - 0.5) ** 2 - 0.8))

# ---- sharding geometry ----
N_CORES = 8
P = 128                      # SBUF partitions
ROW_LO_ALL = -398            # min possible m0 (floor of t), |t| < 397.2
ROWS_PER_CORE = 100
R_ROWS = ROWS_PER_CORE + 5   # 105 slab rows per core (footprint halo)
KU = 804                     # slab cols, c'_u in [-401, 403)
COL_BASE = -401
ROW_F32 = 6464               # padded slab row: 808 cols * 8 = 101*256B stride
STRIP = 408                  # stage-2 ku strip width (2 overlapping strips)
SOFF = (0, 396)              # strip col offsets; windows never straddle
N1 = 3 * R_ROWS + 1          # stage-1 rhs width (f32r needs even N)

NRES = 8                     # col-residue streams per strip
R_SLOTS = 14                 # vis slots per partition per (strip, residue)
N_STREAMS = 2 * NRES         # 16 gather streams
V_SLOTS = N_STREAMS * R_SLOTS        # 224 output rows per partition
GSTRIDE = 3328               # f32 per strip-grid row (52*256B stride)
GBLK = GSTRIDE // 64         # 52 64-f32 blocks per row
DESC_PER_S = P * R_SLOTS * J         # 10752 descriptors per stream
CALL_IDX = 1024                      # dma_gather ring capacity per call
IDXCOLS_S = DESC_PER_S // 16          # 672 int16 cols per stream
GROWS2 = (R_ROWS * GSTRIDE - 56) // 64  # 5459 64-f32 rows addressable

C1 = np.float32(1000.0 * 2.0 * np.pi * DL)   # klambda -> rad/pixel
C2 = np.float32(G / (2.0 * np.pi))           # rad/pixel -> grid coord

_NC_CACHE = {}


def _matmul_dtype():
    return os.environ.get("NUFFT_MM_DTYPE", "float32r")


def build_nc():
    """Build the SPMD Bass program (same program for all 8 cores)."""
    key = _matmul_dtype()
    if key in _NC_CACHE:
        return _NC_CACHE[key]

    import concourse.bacc as bacc
    import concourse.mybir as mybir
    import concourse.tile as tile
    from concourse import library_config
    from contextlib import ExitStack

    f32 = mybir.dt.float32
    i16 = mybir.dt.int16
    mm_dt = getattr(mybir.dt, key)

    nc = bacc.Bacc("TRN2", target_bir_lowering=False, debug=False)

    cube_d = nc.dram_tensor("cube", (NCH, NPIX, NPIX), mm_dt, kind="ExternalInput")
    cvt_d = nc.dram_tensor("cvt", (P, 8, N1), mm_dt, kind="ExternalInput")
    cut_d = nc.dram_tensor("cut", (P, 8, KU), mm_dt, kind="ExternalInput")
    sut_d = nc.dram_tensor("sut", (P, 8, KU), mm_dt, kind="ExternalInput")
    gidx_d = nc.dram_tensor("gidx", (P, N_STREAMS * IDXCOLS_S), i16,
                            kind="ExternalInput")
    w48_d = nc.dram_tensor("w48", (P, V_SLOTS, 48), f32, kind="ExternalInput")
    out_d = nc.dram_tensor("vis_out", (P, V_SLOTS, 8), f32,
                           kind="ExternalOutput")
    grid_d = [nc.dram_tensor(f"gridscratch{i}", (R_ROWS, GSTRIDE), f32)
              for i in range(2)]


    with tile.TileContext(nc) as tc:
        with ExitStack() as s12:
            # one lifetime for all pools: stage-3 tiles must NOT reuse
            # stage-1/2 SBUF zones, else their allocations pick up deps on
            # the tall/grid release (forcing gathers to wait for strip 1)
            const_pool = s12.enter_context(tc.tile_pool(name="const", bufs=1))
            cube_pool = s12.enter_context(tc.tile_pool(name="cube", bufs=3))
            tpool = s12.enter_context(tc.tile_pool(name="tmats", bufs=1))
            cpool = s12.enter_context(tc.tile_pool(name="cstream", bufs=4))
            psum_pool = s12.enter_context(
                tc.tile_pool(name="ps", bufs=8, space="PSUM"))

            cvt_sb = const_pool.tile([P, 8, N1], mm_dt)
            nc.sync.dma_start(cvt_sb[:], cvt_d[:])

            # T storage: (p, chan, term[T1,T2,negT1], xc, r)
            tall = tpool.tile([P, NCH, 3, 8, R_ROWS], mm_dt)

            # ---- stage 1: T^T = cube^T . cvt (accumulate over y chunks) ----
            for c in range(NCH):
                ps = [psum_pool.tile([P, N1], f32, tag="ps",
                                     name=f"ps1_{c}_{i}") for i in range(8)]
                for yc in range(8):
                    cb = cube_pool.tile([P, NPIX], mm_dt, tag="cube")
                    nc.sync.dma_start(cb[:], cube_d[c, yc * P:(yc + 1) * P, :])
                    for xt in range(8):
                        nc.tensor.matmul(
                            ps[xt][:],
                            lhsT=cb[:, xt * P:(xt + 1) * P],
                            rhs=cvt_sb[:, yc, :],
                            start=(yc == 0),
                            stop=(yc == 7),
                        )
                for xt in range(8):
                    for term in range(3):
                        nc.vector.tensor_copy(
                            tall[:, c, term, xt, :],
                            ps[xt][:, term * R_ROWS:(term + 1) * R_ROWS],
                        )

            # ---- stage 2: slab = T . [cut|sut], interleave, DMA to DRAM ----
            grid_sb = tpool.tile([P, KU * 8], f32)
            gv = grid_sb[:].rearrange("p (col e) -> p col e", e=8)
            zpad = cpool.tile([P, GSTRIDE - STRIP * 8], f32, tag="zpad")
            nc.gpsimd.memset(zpad[:], 0.0)
            for strip in range(2):
                off = SOFF[strip]
                ps2 = [psum_pool.tile([P, STRIP], f32, tag="ps",
                                      name=f"ps2_{strip}_{i}")
                       for i in range(8)]  # (c, re/im) -> ps2[c*2+e]
                for xc in range(8):
                    cu = cpool.tile([P, STRIP], mm_dt, tag="cu")
                    nc.sync.dma_start(
                        cu[:], cut_d[:, xc, off:off + STRIP])
                    su = cpool.tile([P, STRIP], mm_dt, tag="su")
                    nc.sync.dma_start(
                        su[:], sut_d[:, xc, off:off + STRIP])
                    for c in range(NCH):
                        t1 = tall[:, c, 0, xc, :]
                        t2 = tall[:, c, 1, xc, :]
                        nt1 = tall[:, c, 2, xc, :]
                        cuv = cu[:]
                        suv = su[:]
                        # re = T1.cu + T2.su ; im = T2.cu + (-T1).su
                        nc.tensor.matmul(ps2[c * 2][:R_ROWS, :], lhsT=t1,
                                         rhs=cuv, start=(xc == 0), stop=False)
                        nc.tensor.matmul(ps2[c * 2][:R_ROWS, :], lhsT=t2,
                                         rhs=suv, start=False, stop=(xc == 7))
                        nc.tensor.matmul(ps2[c * 2 + 1][:R_ROWS, :], lhsT=t2,
                                         rhs=cuv, start=(xc == 0), stop=False)
                        nc.tensor.matmul(ps2[c * 2 + 1][:R_ROWS, :], lhsT=nt1,
                                         rhs=suv, start=False, stop=(xc == 7))
                skip = 0 if strip == 0 else (SOFF[0] + STRIP) - SOFF[1]
                for c in range(NCH):
                    for e in range(2):
                        nc.vector.tensor_copy(
                            gv[:R_ROWS, off + skip:off + STRIP, c * 2 + e],
                            ps2[c * 2 + e][:R_ROWS, skip:],
                        )
                # ship this strip's slab so its gathers can start early
                nc.sync.dma_start(
                    grid_d[strip][:, :STRIP * 8],
                    grid_sb[:R_ROWS, off * 8:(off + STRIP) * 8])
                nc.sync.dma_start(grid_d[strip][:, STRIP * 8:],
                                  zpad[:R_ROWS, :])

            # ---- stage 3: residue-binned dma_gather + weighted reduce ----
            ipool = s12.enter_context(tc.tile_pool(name="interp", bufs=3))
            opool = s12.enter_context(tc.tile_pool(name="outp", bufs=1))

            nc.gpsimd.load_library(library_config.mlp)
            ov = opool.tile([P, V_SLOTS, 8], f32)
            flats = [grid_d[i][:, :].flatten() for i in range(2)]
            for st in range(N_STREAMS):
                sgrid, r = st // NRES, st % NRES
                view_r = flats[sgrid][8 * r: 8 * r + GROWS2 * 64].rearrange(
                    "(n e) -> n e", e=64)
                idxr = ipool.tile([P, IDXCOLS_S], i16, tag="idx",
                                  name=f"idx_{st}")
                nc.sync.dma_start(
                    idxr[:], gidx_d[:, st * IDXCOLS_S:(st + 1) * IDXCOLS_S])
                w = ipool.tile([P, R_SLOTS * 48], f32, tag="w", name=f"w_{st}")
                nc.sync.dma_start(
                    w[:],
                    w48_d[:, st * R_SLOTS:(st + 1) * R_SLOTS, :].rearrange(
                        "p v t -> p (v t)"))
                g = ipool.tile([P, R_SLOTS * J, 64], f32, tag="g",
                               name=f"g_{st}")
                done = 0
                k = 0
                while done < DESC_PER_S:
                    n_idx = min(CALL_IDX, DESC_PER_S - done)
                    nc.gpsimd.dma_gather(
                        out_ap=g[:, done // P:(done + n_idx) // P, :],
                        in_ap=view_r,
                        idxs_ap=idxr[:, done // 16:(done + n_idx) // 16],
                        num_idxs=n_idx,
                        num_idxs_reg=n_idx,
                        elem_size=64,
                        elem_step=64,
                    )
                    done += n_idx
                    k += 1
                # multiply by weights (broadcast over chan/reim)
                gw = g[:].rearrange("p t (col e) -> p (t col) e", e=8)
                wb = w[:].unsqueeze(2).to_broadcast([P, R_SLOTS * 48, 8])
                nc.vector.tensor_tensor(
                    out=gw, in0=gw, in1=wb, op=mybir.AluOpType.mult)
                # reduce over the 48 (6 rows x 8 cols, 2 zero) taps
                rv = g[:].rearrange(
                    "p (v i) (col e) -> p v e (i col)", v=R_SLOTS, i=J, e=8)
                nc.vector.tensor_reduce(
                    out=ov[:, st * R_SLOTS:(st + 1) * R_SLOTS, :],
                    in_=rv,
                    axis=mybir.AxisListType.X,
                    op=mybir.AluOpType.add,
                )
            nc.sync.dma_start(out_d[:], ov[:])

    nc.compile()
    _NC_CACHE[key] = nc
    return nc


def _apod1d():
    f = np.arange(NPIX, dtype=np.float64) / G
    z = np.pi * J * f
    s = np.sqrt(BETA * BETA - z * z)
    return J * np.sinh(s) / s  # [NPIX] float64


def _interp_host(k):
    """Match reference _interp_coords index/weight math in f32."""
    t = (k.astype(np.float32) * C1) * C2
    m0 = np.floor(t).astype(np.int32)
    offs = np.arange(J, dtype=np.int32) - (J // 2 - 1)
    d = t[:, None] - (m0[:, None] + offs).astype(np.float32)
    w = np.i0(BETA * np.sqrt(np.maximum(0.0, 1.0 - (2.0 * d / J) ** 2)))
    return t, m0, w.astype(np.float32)


def host_prep(cube, uu, vv):
    """Returns (in_maps, meta, phase) for the 8 cores."""
    mmkey = _matmul_dtype()
    if mmkey == "bfloat16":
        import ml_dtypes
        mmnp = ml_dtypes.bfloat16
    else:
        mmnp = np.float32
    cube = np.ascontiguousarray(np.asarray(cube, dtype=np.float32)).astype(mmnp)
    uu = np.asarray(uu, dtype=np.float32)
    vv = np.asarray(vv, dtype=np.float32)

    s1 = _apod1d()
    y = np.arange(NPIX, dtype=np.float64)

    # u-direction DFT constants (same for all cores)
    kj = np.arange(KU, dtype=np.float64) + COL_BASE
    ang_u = 2.0 * np.pi * np.outer(y, kj) / G
    cut = (np.cos(ang_u) / s1[:, None]).astype(np.float32)
    sut = (np.sin(ang_u) / s1[:, None]).astype(np.float32)
    cut = np.ascontiguousarray(cut.reshape(8, P, KU).transpose(1, 0, 2)).astype(mmnp)
    sut = np.ascontiguousarray(sut.reshape(8, P, KU).transpose(1, 0, 2)).astype(mmnp)

    tu, m0u, wu = _interp_host(uu)
    tv, m0v, wv = _interp_host(vv)
    assert m0u.min() >= ROW_LO_ALL and m0u.max() < ROW_LO_ALL + 8 * ROWS_PER_CORE
    assert m0v.min() >= ROW_LO_ALL and m0v.max() < ROW_LO_ALL + 8 * ROWS_PER_CORE

    core_of = (m0v - ROW_LO_ALL) // ROWS_PER_CORE
    j0 = m0u - 2 - COL_BASE        # window start col within slab, [1, 796]
    sgrid = (j0 > 400).astype(np.int64)
    colp = j0 - 396 * sgrid        # col within strip grid, [1,400] or [5,407]
    res = colp % NRES
    q = colp // NRES               # 64-f32 block within strip row, [0, 50]
    w48 = np.zeros((len(uu), J, 8), dtype=np.float32)
    w48[:, :, :J] = wv[:, :, None] * wu[:, None, :]

    in_maps = []
    meta = []
    for k in range(N_CORES):
        row_lo = ROW_LO_ALL + ROWS_PER_CORE * k
        gidx = np.zeros((P, N_STREAMS * IDXCOLS_S), dtype=np.int16)
        w48k = np.zeros((P, V_SLOTS, 48), dtype=np.float32)
        meta_k = []
        for st in range(N_STREAMS):
            sg, r = st // NRES, st % NRES
            order = np.where((core_of == k) & (sgrid == sg) & (res == r))[0]
            n = len(order)
            assert n <= P * R_SLOTS, f"core {k} stream {st} overflow: {n}"
            sl = np.arange(n)
            pp = sl % P
            vs = sl // P
            lrow = (m0v[order] - row_lo).astype(np.int64)   # [0, 100)
            vals = (lrow[:, None] + np.arange(J)[None, :]) * GBLK \
                + q[order, None].astype(np.int64)           # [n, J] <= 5458
            # descriptor t = (v*6+i)*128 + p ; idx A[t%16, t//16]
            t = (vs[:, None] * J + np.arange(J)[None, :]) * P + pp[:, None]
            block = np.zeros((16, IDXCOLS_S), dtype=np.int16)
            block[(t % 16).ravel(), (t // 16).ravel()] = vals.astype(
                np.int16).ravel()
            gidx[:, st * IDXCOLS_S:(st + 1) * IDXCOLS_S] = np.tile(block,
                                                                   (8, 1))
            w48k[pp, st * R_SLOTS + vs, :] = w48[order].reshape(n, 48)
            meta_k.append((order, pp, st * R_SLOTS + vs))
        # v-direction DFT constants for this core's rows
        kr = np.arange(R_ROWS, dtype=np.float64) + (row_lo - 2)
        ang_v = 2.0 * np.pi * np.outer(y, kr) / G
        blk = np.zeros((NPIX, 3 * R_ROWS + 1), dtype=np.float32)
        cosb = np.cos(ang_v) / s1[:, None]
        sinb = np.sin(ang_v) / s1[:, None]
        blk[:, 0 * R_ROWS:1 * R_ROWS] = cosb
        blk[:, 1 * R_ROWS:2 * R_ROWS] = -sinb
        blk[:, 2 * R_ROWS:3 * R_ROWS] = -cosb
        cvt = np.ascontiguousarray(
            blk.reshape(8, P, 3 * R_ROWS + 1).transpose(1, 0, 2)).astype(mmnp)

        in_maps.append({
            "cube": cube,
            "cvt": cvt,
            "cut": cut,
            "sut": sut,
            "gidx": gidx,
            "w48": w48k,
        })
        meta.append(meta_k)

    kv = vv * C1
    ku_ = uu * C1
    phase = np.exp(1j * (kv + ku_) * np.float32(NPIX / 2.0)).astype(np.complex64)
    return in_maps, meta, phase


def assemble(results, meta, phase):
    out = np.zeros((NCH, NVIS), dtype=np.complex64)
    for k in range(N_CORES):
        arr = results[k]["vis_out"].reshape(P, V_SLOTS, NCH, 2)
        for order, pp, rows in meta[k]:
            vals = arr[pp, rows]  # [n, NCH, 2]
            out[:, order] = (vals[..., 0] + 1j * vals[..., 1]).T
    return out * phase[None, :]


def kernel(cube, uu, vv):
    from concourse.bass_utils import run_bass_kernel_spmd

    nc = build_nc()
    in_maps, meta, phase = host_prep(cube, uu, vv)
    br = run_bass_kernel_spmd(
        nc, in_maps, list(range(N_CORES)),
        trace=bool(int(os.environ.get("NUFFT_TRACE", "0"))),
    )
    if br.exec_time_ns is not None:
        print(f"HW exec time: {br.exec_time_ns} ns")
    kernel.last_result = br
    return assemble(br.results, meta, phase)



# revision 31
# speedup vs baseline: 1.5078x; 1.5078x over previous
"""NuFFT forward (KbNufft-style) Trainium2 Bass kernel, v2.

Strategy (per core, 8-way SPMD over the visibility v-row bins):
  - DFT-slab: each core computes a 105-row x 824-col patch of the 2048^2
    oversampled spectrum via two matmul stages (apodization folded in):
        T = cube^T . cvt            (per chan: [1024x, 210] = [T1|T2])
        slab = T^T . [cut|sut|nsut] (105 rows x 824 cols x 4chan x re/im)
    in fp16 (f32 PSUM accumulate; constants carry a 2^8 range scale each,
    undone in the f32 weight table -- fp16's 10-bit mantissa is needed
    because slab quantization error is ~17x amplified by the KB weighted
    sum's cancellation against the corner-placed image's phase ramp).
  - Row-stacked interp grid: E[r][j][m][e] fp16 in DRAM stores, for every
    footprint start row r, the 6 consecutive slab rows r..r+5 — so one
    visibility's whole 6-row x 8-col KB footprint is ONE contiguous 768B
    gather element (full DMA bus rate, 1 descriptor/vis instead of 6).
  - Visibilities binned by (col strip, element-offset residue u%8) into 16
    gather streams; a DVE multiply(+)reduce against 36-tap weight products
    (6x6 window read via a stride-8 contiguous slice) yields the outputs.
"""
import os
import sys

for _p in ("/opt/trn_rl_repo",):
    if _p not in sys.path and os.path.isdir(_p):
        sys.path.insert(0, _p)

import numpy as np

# ---- problem constants (must match reference.py) ----
NCH = 4
NPIX = 1024
NVIS = 200_000
G = 2048
J = 6
OSF = 2
CELL_ARCSEC = 0.005
DL = CELL_ARCSEC * np.pi / (180.0 * 3600.0)
BETA = float(np.pi * np.sqrt((J / OSF) ** 2 * (OSF - 0.5) ** 2 - 0.8))

# ---- geometry ----
N_CORES = 8
P = 128
ROW0 = -400            # j0/r0 global offset: r_g = (m0v-2) + 400 in [0, 796)
RPC = 100              # E rows (footprint starts) per core
R_ROWS = 105           # slab rows per core (RPC + 5 halo)
N1 = 210               # stage-1 rhs width: [T1 | T2]
KU = 824               # total slab cols, spectrum col = j - 400
SW = 416               # col strip width
SOFF = (0, 408)        # strip col offsets
JSPLIT = 408           # j0_global <= 408 -> strip 0, else strip 1
NRES = 8               # element-offset residues (u % 8) per strip
R_SLOTS = 14           # vis slots per partition per stream
N_STREAMS = 2 * NRES
V_SLOTS = N_STREAMS * R_SLOTS          # 224 slots per partition
ELEM = 48 * 8          # gather element: 8 cols x 6 rows x 8 (chan,re/im) fp16
IDX_PER_S = P * R_SLOTS                # 1792 descriptors per stream
IDXCOLS = IDX_PER_S // 16              # 112 int16 cols per stream
EVIEW_N = 5200         # gather view rows (max idx 5199)
E_BODY = RPC * SW * 48                 # 1,996,800 bf16 per strip grid
E_FLAT = E_BODY + 2 * ELEM             # + tail pad for the offset views
CHW = SW // 2          # E-build column chunk width (208)
NCHUNK = SW // CHW

C1 = np.float32(1000.0 * 2.0 * np.pi * DL)   # klambda -> rad/pixel
C2 = np.float32(G / (2.0 * np.pi))           # rad/pixel -> grid coord

_NC_CACHE = {}


def build_nc():
    """Build the SPMD Bass program (same program for all 8 cores)."""
    if "nc" in _NC_CACHE:
        return _NC_CACHE["nc"]

    import concourse.bacc as bacc
    import concourse.mybir as mybir
    import concourse.tile as tile
    from concourse import library_config
    from contextlib import ExitStack

    f32 = mybir.dt.float32
    f16 = mybir.dt.float16
    i16 = mybir.dt.int16

    nc = bacc.Bacc("TRN2", target_bir_lowering=False, debug=False,
                   dynamic_dma_scratch_size=16384)

    cube_d = nc.dram_tensor("cube", (NCH, NPIX, NPIX), f16, kind="ExternalInput")
    cvt_d = nc.dram_tensor("cvt", (P, 8, N1), f16, kind="ExternalInput")
    cut_d = nc.dram_tensor("cut", (P, 8, KU), f16, kind="ExternalInput")
    sut_d = nc.dram_tensor("sut", (P, 8, KU), f16, kind="ExternalInput")
    nsut_d = nc.dram_tensor("nsut", (P, 8, KU), f16, kind="ExternalInput")
    gidx_d = nc.dram_tensor("gidx", (P, N_STREAMS * IDXCOLS), i16,
                            kind="ExternalInput")
    w36_d = nc.dram_tensor("w36", (P, V_SLOTS, 36), f32, kind="ExternalInput")
    out_d = nc.dram_tensor("vis_out", (P, V_SLOTS, 8), f32,
                           kind="ExternalOutput")
    e_d = [nc.dram_tensor(f"egrid{i}", (1, E_FLAT), f16) for i in range(2)]

    with tile.TileContext(nc) as tc:
        with ExitStack() as s12:
            const_pool = s12.enter_context(tc.tile_pool(name="const", bufs=1))
            cube_pool = s12.enter_context(tc.tile_pool(name="cube", bufs=2))
            tpool = s12.enter_context(tc.tile_pool(name="tmats", bufs=1))
            cpool = s12.enter_context(tc.tile_pool(name="cstream", bufs=1))
            spool = s12.enter_context(tc.tile_pool(name="slab", bufs=1))
            epool = s12.enter_context(tc.tile_pool(name="ebuild", bufs=1))
            etpool = s12.enter_context(tc.tile_pool(name="etmp2", bufs=2))
            psum_pool = s12.enter_context(
                tc.tile_pool(name="ps", bufs=1, space="PSUM"))

            cvt_sb = const_pool.tile([P, 8, N1], f16)
            nc.sync.dma_start(cvt_sb[:], cvt_d[:])
            idx_sb = const_pool.tile([P, N_STREAMS * IDXCOLS], i16)
            w36_sb = const_pool.tile([P, V_SLOTS, 36], f32)

            # T storage: (p, chan, term[T1,T2], xc, r)
            tall = tpool.tile([P, NCH, 2, 8, R_ROWS], f16)

            # zero the E-grid tail pads up front (keeps the gather views
            # finite without sitting in the E-build critical chain)
            zp = spool.tile([P, (E_FLAT - E_BODY) // P], f16, tag="zp")
            nc.gpsimd.memset(zp[:], 0.0)
            for i in range(2):
                nc.scalar.dma_start(
                    e_d[i][0, E_BODY:].rearrange("(p x) -> p x", p=P), zp[:])

            # ---- stage 1: T^T = cube^T . cvt (accumulate over y chunks) ----
            # one 8-bank PSUM tile; bank xt holds column block xt's accum
            for c in range(NCH):
                psA = psum_pool.tile([P, 8, 512], f32, tag="ps",
                                     name=f"ps1_{c}")
                for h in range(2):
                    cb = cube_pool.tile([P, 4, NPIX], f16, tag="cube")
                    nc.sync.dma_start(
                        cb[:], cube_d[c, h * 4 * P:(h + 1) * 4 * P, :]
                        .rearrange("(k p) x -> p k x", p=P))
                    for k in range(4):
                        yc = h * 4 + k
                        for xt in range(8):
                            nc.tensor.matmul(
                                psA[:, xt, :N1],
                                lhsT=cb[:, k, xt * P:(xt + 1) * P],
                                rhs=cvt_sb[:, yc, :],
                                start=(yc == 0),
                                stop=(yc == 7),
                            )
                nc.scalar.activation(
                    tall[:, c, :, :, :],
                    psA[:, :, :N1].rearrange("p xt (t r) -> p t xt r", t=2),
                    mybir.ActivationFunctionType.Copy,
                )

            # ---- per strip: stage 2, E-build, gather, weighted reduce ----
            nc.gpsimd.load_library(library_config.mlp)
            opool = s12.enter_context(tc.tile_pool(name="outp", bufs=1))
            ipool = s12.enter_context(tc.tile_pool(name="interp", bufs=2))
            gopool = s12.enter_context(tc.tile_pool(name="gout", bufs=1))
            ov = opool.tile([P, V_SLOTS, 8], f32)

            cusu_next = None
            for strip in range(2):
                off = SOFF[strip]
                ps2 = psum_pool.tile([P, 8, 512], f32, tag="ps",
                                     name=f"ps2_{strip}")  # bank = c*2+e
                if strip == 0:
                    cus = cpool.tile([P, 8, SW], f16, tag="cu")
                    nc.sync.dma_start(cus[:], cut_d[:, :, off:off + SW])
                    sus = cpool.tile([P, 8, SW], f16, tag="su")
                    nc.sync.dma_start(sus[:], sut_d[:, :, off:off + SW])
                    nsus = cpool.tile([P, 8, SW], f16, tag="nsu")
                    nc.sync.dma_start(nsus[:], nsut_d[:, :, off:off + SW])
                    # interp tables on the Pool queue: it is idle until the
                    # first gathers, so these never block tall copies/cube
                    nc.gpsimd.dma_start(idx_sb[:], gidx_d[:])
                    nc.gpsimd.dma_start(w36_sb[:], w36_d[:])
                else:
                    cus, sus, nsus = cusu_next
                for c in range(NCH):
                    for xc in range(8):
                        cu = cus[:, xc, :]
                        su = sus[:, xc, :]
                        nsu = nsus[:, xc, :]
                        t1 = tall[:, c, 0, xc, :]
                        t2 = tall[:, c, 1, xc, :]
                        # re = T1.cu + T2.su ; im = T2.cu + T1.(-su)
                        nc.tensor.matmul(ps2[:R_ROWS, c * 2, :SW], lhsT=t1,
                                         rhs=cu, start=(xc == 0), stop=False)
                        nc.tensor.matmul(ps2[:R_ROWS, c * 2, :SW], lhsT=t2,
                                         rhs=su, start=False, stop=(xc == 7))
                        nc.tensor.matmul(ps2[:R_ROWS, c * 2 + 1, :SW], lhsT=t2,
                                         rhs=cu, start=(xc == 0), stop=False)
                        nc.tensor.matmul(ps2[:R_ROWS, c * 2 + 1, :SW], lhsT=t1,
                                         rhs=nsu, start=False, stop=(xc == 7))
                slab = spool.tile([P, SW, 8], f16, tag="slab")
                nc.scalar.activation(
                    slab[:R_ROWS],
                    ps2[:R_ROWS, :, :SW].rearrange("p ce j -> p j ce"),
                    mybir.ActivationFunctionType.Copy)

                # E-build: stack rows r..r+5 contiguously, per column chunk
                ev = e_d[strip][0, :E_BODY].rearrange(
                    "(r c x) -> r c x", c=NCHUNK, x=CHW * 48)
                for ch in range(NCHUNK):
                    etmp = etpool.tile([P, 6, CHW, 8], f16, tag="etmp")
                    for m in range(6):
                        eng = (nc.sync, nc.scalar)[m % 2]
                        eng.dma_start(
                            etmp[:RPC, m, :, :],
                            slab[m:m + RPC, ch * CHW:(ch + 1) * CHW, :])
                    ero = epool.tile([P, CHW, 6, 8], f16, tag="ero")
                    if strip == 0:
                        nc.vector.tensor_copy(
                            ero[:RPC],
                            etmp[:RPC].rearrange("p m j e -> p j m e"))
                    else:
                        nc.scalar.activation(
                            ero[:RPC],
                            etmp[:RPC].rearrange("p m j e -> p j m e"),
                            mybir.ActivationFunctionType.Copy)
                    nc.sync.dma_start(
                        ev[:, ch, :],
                        ero[:RPC].rearrange("p j m e -> p (j m e)"))

                if strip == 0:
                    # strip-1 constants on the in-order Act queue, emitted
                    # after the E0-build so they cannot jump ahead of its
                    # shifts on the DMA engines
                    o1 = SOFF[1]
                    cusu_next = []
                    for nm, td in (("cu", cut_d), ("su", sut_d),
                                   ("nsu", nsut_d)):
                        t = cpool.tile([P, 8, SW], f16, tag=nm)
                        nc.scalar.dma_start(t[:], td[:, :, o1:o1 + SW])
                        cusu_next.append(t)

                # gather + weighted reduce, one stream per residue
                for rr in range(NRES):
                    st = strip * NRES + rr
                    g = ipool.tile([P, R_SLOTS, ELEM], f16, tag="g",
                                   name=f"g_{st}")
                    view = e_d[strip][0, 48 * rr:48 * rr + EVIEW_N * ELEM]
                    view = view.rearrange("(n e) -> n e", e=ELEM)
                    done = 0
                    while done < IDX_PER_S:
                        n_idx = min(1024, IDX_PER_S - done)  # SWDGE ring cap
                        nc.gpsimd.dma_gather(
                            out_ap=g[:, done // P:(done + n_idx) // P, :],
                            in_ap=view,
                            idxs_ap=idx_sb[:, st * IDXCOLS + done // 16:
                                           st * IDXCOLS + (done + n_idx) // 16],
                            num_idxs=n_idx,
                            num_idxs_reg=n_idx,
                            elem_size=ELEM,
                            elem_step=ELEM,
                        )
                        done += n_idx
                    # multiply by the 36 window-tap weights (f32 products)
                    gv = g[:].rearrange("p v (t e) -> p v t e", e=8)
                    gv = gv[:, :, :36, :]
                    wb = w36_sb[:, st * R_SLOTS:(st + 1) * R_SLOTS, :]
                    wb = wb.unsqueeze(3).to_broadcast([P, R_SLOTS, 36, 8])
                    gout = gopool.tile([P, R_SLOTS, 36, 8], f32, tag="gout",
                                       name=f"gout_{st}")
                    nc.vector.tensor_tensor(
                        out=gout[:], in0=gv, in1=wb, op=mybir.AluOpType.mult)
                    rv = gout[:].rearrange("p v t e -> p v e t")
                    nc.vector.tensor_reduce(
                        out=ov[:, st * R_SLOTS:(st + 1) * R_SLOTS, :],
                        in_=rv,
                        axis=mybir.AxisListType.X,
                        op=mybir.AluOpType.add,
                    )
                nc.sync.dma_start(
                    out_d[:, strip * 8 * R_SLOTS:(strip + 1) * 8 * R_SLOTS, :],
                    ov[:, strip * 8 * R_SLOTS:(strip + 1) * 8 * R_SLOTS, :])

    nc.compile()
    _NC_CACHE["nc"] = nc
    return nc


def _apod1d():
    f = np.arange(NPIX, dtype=np.float64) / G
    z = np.pi * J * f
    s = np.sqrt(BETA * BETA - z * z)
    return J * np.sinh(s) / s  # [NPIX] float64


def _interp_host(k):
    """Match reference _interp_coords index/weight math in f32."""
    t = (k.astype(np.float32) * C1) * C2
    m0 = np.floor(t).astype(np.int32)
    offs = np.arange(J, dtype=np.int32) - (J // 2 - 1)
    d = t[:, None] - (m0[:, None] + offs).astype(np.float32)
    w = np.i0(BETA * np.sqrt(np.maximum(0.0, 1.0 - (2.0 * d / J) ** 2)))
    return t, m0, w.astype(np.float32)


SC = 256.0        # fp16 range scale for each DFT constant family
WDESC = 1.0 / (SC * SC)   # weight descale: slab carries SC^2


def host_prep(cube, uu, vv):
    """Returns (in_maps, meta, phase) for the 8 cores."""
    f16 = np.float16
    cube = np.ascontiguousarray(np.asarray(cube, dtype=np.float32)).astype(f16)
    uu = np.asarray(uu, dtype=np.float32)
    vv = np.asarray(vv, dtype=np.float32)

    s1 = _apod1d()
    y = np.arange(NPIX, dtype=np.float64)

    # u-direction DFT constants (same for all cores)
    kj = np.arange(KU, dtype=np.float64) + ROW0
    ang_u = 2.0 * np.pi * np.outer(y, kj) / G
    cut = SC * np.cos(ang_u) / s1[:, None]
    sut = SC * np.sin(ang_u) / s1[:, None]

    def fold(a):
        return np.ascontiguousarray(
            a.reshape(8, P, KU).transpose(1, 0, 2)).astype(np.float16)

    cutf, sutf, nsutf = fold(cut), fold(sut), fold(-sut)

    tu, m0u, wu = _interp_host(uu)
    tv, m0v, wv = _interp_host(vv)
    rg = m0v - 2 - ROW0                # [0, 796)
    jg = m0u - 2 - ROW0                # [0, 796)
    assert rg.min() >= 0 and rg.max() < N_CORES * RPC
    assert jg.min() >= 0 and jg.max() + 7 < KU

    core_of = rg // RPC
    r = rg - core_of * RPC             # E row within core, [0, 100)
    sgrid = (jg > JSPLIT).astype(np.int64)
    j0loc = jg - np.array(SOFF)[sgrid]
    assert j0loc.min() >= 0 and (j0loc + 7).max() < SW
    u = r * SW + j0loc
    res = u % NRES
    idx = u // NRES
    assert idx.max() < EVIEW_N

    w36 = (wu[:, :, None] * wv[:, None, :]) * WDESC   # [n, c, m] tap products

    in_maps = []
    meta = []
    for k in range(N_CORES):
        gidx = np.zeros((P, N_STREAMS * IDXCOLS), dtype=np.int16)
        w36k = np.zeros((P, V_SLOTS, 36), dtype=np.float32)
        meta_k = []
        for st in range(N_STREAMS):
            sg, rr = st // NRES, st % NRES
            order = np.where((core_of == k) & (sgrid == sg) & (res == rr))[0]
            n = len(order)
            assert n <= IDX_PER_S, f"core {k} stream {st} overflow: {n}"
            sl = np.arange(n)
            pp = sl % P
            vs = sl // P
            t = sl  # descriptor index == slot index
            block = np.zeros((16, IDXCOLS), dtype=np.int16)
            block[t % 16, t // 16] = idx[order].astype(np.int16)
            gidx[:, st * IDXCOLS:(st + 1) * IDXCOLS] = np.tile(block, (8, 1))
            w36k[pp, st * R_SLOTS + vs, :] = w36[order].reshape(n, 36)
            meta_k.append((order, pp, st * R_SLOTS + vs))
        # v-direction DFT constants for this core's slab rows
        kr = np.arange(R_ROWS, dtype=np.float64) + (ROW0 + RPC * k)
        ang_v = 2.0 * np.pi * np.outer(y, kr) / G
        blk = np.empty((NPIX, N1), dtype=np.float64)
        blk[:, :R_ROWS] = SC * np.cos(ang_v) / s1[:, None]
        blk[:, R_ROWS:] = -SC * np.sin(ang_v) / s1[:, None]
        cvt = np.ascontiguousarray(
            blk.reshape(8, P, N1).transpose(1, 0, 2)).astype(np.float16)

        in_maps.append({
            "cube": cube,
            "cvt": cvt,
            "cut": cutf,
            "sut": sutf,
            "nsut": nsutf,
            "gidx": gidx,
            "w36": w36k,
        })
        meta.append(meta_k)

    kv = vv * C1
    ku_ = uu * C1
    phase = np.exp(1j * (kv + ku_) * np.float32(NPIX / 2.0)).astype(np.complex64)
    return in_maps, meta, phase


def assemble(results, meta, phase):
    out = np.zeros((NCH, NVIS), dtype=np.complex64)
    for k in range(N_CORES):
        arr = results[k]["vis_out"].reshape(P, V_SLOTS, NCH, 2)
        for order, pp, rows in meta[k]:
            vals = arr[pp, rows]  # [n, NCH, 2]
            out[:, order] = (vals[..., 0] + 1j * vals[..., 1]).T
    return out * phase[None, :]


def kernel(cube, uu, vv):
    from concourse.bass_utils import run_bass_kernel_spmd

    nc = build_nc()
    in_maps, meta, phase = host_prep(cube, uu, vv)
    br = run_bass_kernel_spmd(
        nc, in_maps, list(range(N_CORES)),
        trace=bool(int(os.environ.get("NUFFT_TRACE", "0"))),
    )
    if br.exec_time_ns is not None:
        print(f"HW exec time: {br.exec_time_ns} ns")
    kernel.last_result = br
    return assemble(br.results, meta, phase)


# revision 37
# speedup vs baseline: 1.5714x; 1.0422x over previous
"""NuFFT forward (KbNufft-style) Trainium2 Bass kernel, v2.

Strategy (per core, 8-way SPMD over the visibility v-row bins):
  - DFT-slab: each core computes a 105-row x 824-col patch of the 2048^2
    oversampled spectrum via two matmul stages (apodization folded in):
        T = cube^T . cvt            (per chan: [1024x, 210] = [T1|T2])
        slab = T^T . [cut|sut|nsut] (105 rows x 824 cols x 4chan x re/im)
    in fp16 (f32 PSUM accumulate; constants carry a 2^8 range scale each,
    undone in the f32 weight table -- fp16's 10-bit mantissa is needed
    because slab quantization error is ~17x amplified by the KB weighted
    sum's cancellation against the corner-placed image's phase ramp).
  - Row-stacked interp grid: E[r][j][m][e] fp16 in DRAM stores, for every
    footprint start row r, the 6 consecutive slab rows r..r+5 — so one
    visibility's whole 6-row x 8-col KB footprint is ONE contiguous 768B
    gather element (full DMA bus rate, 1 descriptor/vis instead of 6).
  - Visibilities binned by (col strip, element-offset residue u%8) into 16
    gather streams; a DVE multiply(+)reduce against 36-tap weight products
    (6x6 window read via a stride-8 contiguous slice) yields the outputs.
"""
import os
import sys

for _p in ("/opt/trn_rl_repo",):
    if _p not in sys.path and os.path.isdir(_p):
        sys.path.insert(0, _p)

import numpy as np

# ---- problem constants (must match reference.py) ----
NCH = 4
NPIX = 1024
NVIS = 200_000
G = 2048
J = 6
OSF = 2
CELL_ARCSEC = 0.005
DL = CELL_ARCSEC * np.pi / (180.0 * 3600.0)
BETA = float(np.pi * np.sqrt((J / OSF) ** 2 * (OSF - 0.5) ** 2 - 0.8))

# ---- geometry ----
N_CORES = 8
P = 128
ROW0 = -400            # j0/r0 global offset: r_g = (m0v-2) + 400 in [0, 796)
RPC = 100              # E rows (footprint starts) per core
R_ROWS = 105           # slab rows per core (RPC + 5 halo)
N1 = 210               # stage-1 rhs width: [T1 | T2]
KU = 824               # total slab cols, spectrum col = j - 400
SW = 416               # col strip width
SOFF = (0, 388)        # strip col offsets (overlap cols 388..408 flexible)
JHI = 408              # strip 0 covers j0 <= 408; strip 1 covers j0 >= 396
NRES = 8               # element-offset residues (u % 8) per strip
R_SLOTS = 13           # vis slots per partition per stream
N_STREAMS = 2 * NRES
V_SLOTS = N_STREAMS * R_SLOTS          # 224 slots per partition
ELEM = 48 * 8          # gather element: 8 cols x 6 rows x 8 (chan,re/im) fp16
IDX_PER_S = P * R_SLOTS                # 1792 descriptors per stream
IDXCOLS = IDX_PER_S // 16              # 112 int16 cols per stream
EVIEW_N = 5200         # gather view rows (max idx 5199)
E_BODY = RPC * SW * 48                 # 1,996,800 bf16 per strip grid
E_FLAT = E_BODY + 2 * ELEM             # + tail pad for the offset views
CHW = SW // 2          # E-build column chunk width (208)
NCHUNK = SW // CHW

C1 = np.float32(1000.0 * 2.0 * np.pi * DL)   # klambda -> rad/pixel
C2 = np.float32(G / (2.0 * np.pi))           # rad/pixel -> grid coord

_NC_CACHE = {}


def build_nc():
    """Build the SPMD Bass program (same program for all 8 cores)."""
    if "nc" in _NC_CACHE:
        return _NC_CACHE["nc"]

    import concourse.bacc as bacc
    import concourse.mybir as mybir
    import concourse.tile as tile
    from concourse import library_config
    from contextlib import ExitStack

    f32 = mybir.dt.float32
    f16 = mybir.dt.float16
    i16 = mybir.dt.int16

    nc = bacc.Bacc("TRN2", target_bir_lowering=False, debug=False,
                   dynamic_dma_scratch_size=16384)

    cube_d = nc.dram_tensor("cube", (NCH, NPIX, NPIX), f16, kind="ExternalInput")
    cvt_d = nc.dram_tensor("cvt", (P, 8, N1), f16, kind="ExternalInput")
    cut_d = nc.dram_tensor("cut", (P, 8, KU), f16, kind="ExternalInput")
    sut_d = nc.dram_tensor("sut", (P, 8, KU), f16, kind="ExternalInput")
    nsut_d = nc.dram_tensor("nsut", (P, 8, KU), f16, kind="ExternalInput")
    gidx_d = nc.dram_tensor("gidx", (P, N_STREAMS * IDXCOLS), i16,
                            kind="ExternalInput")
    w36_d = nc.dram_tensor("w36", (P, V_SLOTS, 36), f32, kind="ExternalInput")
    out_d = nc.dram_tensor("vis_out", (P, V_SLOTS, 8), f32,
                           kind="ExternalOutput")
    e_d = [nc.dram_tensor(f"egrid{i}", (1, E_FLAT), f16) for i in range(2)]

    with tile.TileContext(nc) as tc:
        with ExitStack() as s12:
            const_pool = s12.enter_context(tc.tile_pool(name="const", bufs=1))
            cube_pool = s12.enter_context(tc.tile_pool(name="cube", bufs=2))
            tpool = s12.enter_context(tc.tile_pool(name="tmats", bufs=1))
            cpool = s12.enter_context(tc.tile_pool(name="cstream", bufs=1))
            spool = s12.enter_context(tc.tile_pool(name="slab", bufs=1))
            epool = s12.enter_context(tc.tile_pool(name="ebuild", bufs=1))
            etpool = s12.enter_context(tc.tile_pool(name="etmp2", bufs=2))
            psum_pool = s12.enter_context(
                tc.tile_pool(name="ps", bufs=1, space="PSUM"))

            cvt_sb = const_pool.tile([P, 8, N1], f16)
            nc.sync.dma_start(cvt_sb[:], cvt_d[:])
            idx_sb = const_pool.tile([P, N_STREAMS * IDXCOLS], i16)
            w36_sb = const_pool.tile([P, V_SLOTS, 36], f32)

            # T storage: (p, chan, term[T1,T2], xc, r)
            tall = tpool.tile([P, NCH, 2, 8, R_ROWS], f16)

            # zero the E-grid tail pads up front (keeps the gather views
            # finite without sitting in the E-build critical chain)
            zp = spool.tile([P, (E_FLAT - E_BODY) // P], f16, tag="zp")
            nc.gpsimd.memset(zp[:], 0.0)
            for i in range(2):
                nc.scalar.dma_start(
                    e_d[i][0, E_BODY:].rearrange("(p x) -> p x", p=P), zp[:])

            # ---- stage 1: T^T = cube^T . cvt (accumulate over y chunks) ----
            # one 8-bank PSUM tile; bank xt holds column block xt's accum
            for c in range(NCH):
                psA = psum_pool.tile([P, 8, 512], f32, tag="ps",
                                     name=f"ps1_{c}")
                for h in range(2):
                    cb = cube_pool.tile([P, 4, NPIX], f16, tag="cube")
                    nc.sync.dma_start(
                        cb[:], cube_d[c, h * 4 * P:(h + 1) * 4 * P, :]
                        .rearrange("(k p) x -> p k x", p=P))
                    for k in range(4):
                        yc = h * 4 + k
                        for xt in range(8):
                            nc.tensor.matmul(
                                psA[:, xt, :N1],
                                lhsT=cb[:, k, xt * P:(xt + 1) * P],
                                rhs=cvt_sb[:, yc, :],
                                start=(yc == 0),
                                stop=(yc == 7),
                            )
                nc.scalar.activation(
                    tall[:, c, :, :, :],
                    psA[:, :, :N1].rearrange("p xt (t r) -> p t xt r", t=2),
                    mybir.ActivationFunctionType.Copy,
                )

            # ---- per strip: stage 2, E-build, gather, weighted reduce ----
            nc.gpsimd.load_library(library_config.mlp)
            opool = s12.enter_context(tc.tile_pool(name="outp", bufs=1))
            ipool = s12.enter_context(tc.tile_pool(name="interp", bufs=2))
            gopool = s12.enter_context(tc.tile_pool(name="gout", bufs=1))
            ov = opool.tile([P, V_SLOTS, 8], f32)

            cusu_next = None
            for strip in range(2):
                off = SOFF[strip]
                ps2 = psum_pool.tile([P, 8, 512], f32, tag="ps",
                                     name=f"ps2_{strip}")  # bank = c*2+e
                if strip == 0:
                    cus = cpool.tile([P, 8, SW], f16, tag="cu")
                    nc.sync.dma_start(cus[:], cut_d[:, :, off:off + SW])
                    sus = cpool.tile([P, 8, SW], f16, tag="su")
                    nc.sync.dma_start(sus[:], sut_d[:, :, off:off + SW])
                    nsus = cpool.tile([P, 8, SW], f16, tag="nsu")
                    nc.sync.dma_start(nsus[:], nsut_d[:, :, off:off + SW])
                    # interp tables on the Pool queue: it is idle until the
                    # first gathers, so these never block tall copies/cube
                    nc.gpsimd.dma_start(idx_sb[:], gidx_d[:])
                    nc.gpsimd.dma_start(w36_sb[:], w36_d[:])
                else:
                    cus, sus, nsus = cusu_next
                for c in range(NCH):
                    for xc in range(8):
                        cu = cus[:, xc, :]
                        su = sus[:, xc, :]
                        nsu = nsus[:, xc, :]
                        t1 = tall[:, c, 0, xc, :]
                        t2 = tall[:, c, 1, xc, :]
                        # re = T1.cu + T2.su ; im = T2.cu + T1.(-su)
                        nc.tensor.matmul(ps2[:R_ROWS, c * 2, :SW], lhsT=t1,
                                         rhs=cu, start=(xc == 0), stop=False)
                        nc.tensor.matmul(ps2[:R_ROWS, c * 2, :SW], lhsT=t2,
                                         rhs=su, start=False, stop=(xc == 7))
                        nc.tensor.matmul(ps2[:R_ROWS, c * 2 + 1, :SW], lhsT=t2,
                                         rhs=cu, start=(xc == 0), stop=False)
                        nc.tensor.matmul(ps2[:R_ROWS, c * 2 + 1, :SW], lhsT=t1,
                                         rhs=nsu, start=False, stop=(xc == 7))
                slab = spool.tile([P, SW, 8], f16, tag="slab")
                nc.scalar.activation(
                    slab[:R_ROWS],
                    ps2[:R_ROWS, :, :SW].rearrange("p ce j -> p j ce"),
                    mybir.ActivationFunctionType.Copy)

                # E-build: stack rows r..r+5 contiguously, per column chunk
                ev = e_d[strip][0, :E_BODY].rearrange(
                    "(r c x) -> r c x", c=NCHUNK, x=CHW * 48)
                for ch in range(NCHUNK):
                    etmp = etpool.tile([P, 6, CHW, 8], f16, tag="etmp")
                    for m in range(6):
                        eng = (nc.sync, nc.scalar)[m % 2]
                        eng.dma_start(
                            etmp[:RPC, m, :, :],
                            slab[m:m + RPC, ch * CHW:(ch + 1) * CHW, :])
                    ero = epool.tile([P, CHW, 6, 8], f16, tag="ero")
                    if strip == 0:
                        nc.vector.tensor_copy(
                            ero[:RPC],
                            etmp[:RPC].rearrange("p m j e -> p j m e"))
                    else:
                        nc.scalar.activation(
                            ero[:RPC],
                            etmp[:RPC].rearrange("p m j e -> p j m e"),
                            mybir.ActivationFunctionType.Copy)
                    nc.sync.dma_start(
                        ev[:, ch, :],
                        ero[:RPC].rearrange("p j m e -> p (j m e)"))

                if strip == 0:
                    # strip-1 constants on the in-order Act queue, emitted
                    # after the E0-build so they cannot jump ahead of its
                    # shifts on the DMA engines
                    o1 = SOFF[1]
                    cusu_next = []
                    for nm, td in (("cu", cut_d), ("su", sut_d),
                                   ("nsu", nsut_d)):
                        t = cpool.tile([P, 8, SW], f16, tag=nm)
                        nc.scalar.dma_start(t[:], td[:, :, o1:o1 + SW])
                        cusu_next.append(t)

                # gather + weighted reduce, one stream per residue
                for rr in range(NRES):
                    st = strip * NRES + rr
                    g = ipool.tile([P, R_SLOTS, ELEM], f16, tag="g",
                                   name=f"g_{st}")
                    view = e_d[strip][0, 48 * rr:48 * rr + EVIEW_N * ELEM]
                    view = view.rearrange("(n e) -> n e", e=ELEM)
                    done = 0
                    while done < IDX_PER_S:
                        n_idx = min(1024, IDX_PER_S - done)  # SWDGE ring cap
                        nc.gpsimd.dma_gather(
                            out_ap=g[:, done // P:(done + n_idx) // P, :],
                            in_ap=view,
                            idxs_ap=idx_sb[:, st * IDXCOLS + done // 16:
                                           st * IDXCOLS + (done + n_idx) // 16],
                            num_idxs=n_idx,
                            num_idxs_reg=n_idx,
                            elem_size=ELEM,
                            elem_step=ELEM,
                        )
                        done += n_idx
                    # multiply by the 36 window-tap weights (f32 products)
                    gv = g[:].rearrange("p v (t e) -> p v t e", e=8)
                    gv = gv[:, :, :36, :]
                    wb = w36_sb[:, st * R_SLOTS:(st + 1) * R_SLOTS, :]
                    wb = wb.unsqueeze(3).to_broadcast([P, R_SLOTS, 36, 8])
                    gout = gopool.tile([P, R_SLOTS, 36, 8], f32, tag="gout",
                                       name=f"gout_{st}")
                    nc.vector.tensor_tensor(
                        out=gout[:], in0=gv, in1=wb, op=mybir.AluOpType.mult)
                    rv = gout[:].rearrange("p v t e -> p v e t")
                    nc.vector.tensor_reduce(
                        out=ov[:, st * R_SLOTS:(st + 1) * R_SLOTS, :],
                        in_=rv,
                        axis=mybir.AxisListType.X,
                        op=mybir.AluOpType.add,
                    )
                nc.sync.dma_start(
                    out_d[:, strip * 8 * R_SLOTS:(strip + 1) * 8 * R_SLOTS, :],
                    ov[:, strip * 8 * R_SLOTS:(strip + 1) * 8 * R_SLOTS, :])

    nc.compile()
    _NC_CACHE["nc"] = nc
    return nc


def _apod1d():
    f = np.arange(NPIX, dtype=np.float64) / G
    z = np.pi * J * f
    s = np.sqrt(BETA * BETA - z * z)
    return J * np.sinh(s) / s  # [NPIX] float64


def _interp_host(k):
    """Match reference _interp_coords index/weight math in f32."""
    t = (k.astype(np.float32) * C1) * C2
    m0 = np.floor(t).astype(np.int32)
    offs = np.arange(J, dtype=np.int32) - (J // 2 - 1)
    d = t[:, None] - (m0[:, None] + offs).astype(np.float32)
    w = np.i0(BETA * np.sqrt(np.maximum(0.0, 1.0 - (2.0 * d / J) ** 2)))
    return t, m0, w.astype(np.float32)


SC = 256.0        # fp16 range scale for each DFT constant family
WDESC = 1.0 / (SC * SC)   # weight descale: slab carries SC^2


def host_prep(cube, uu, vv):
    """Returns (in_maps, meta, phase) for the 8 cores."""
    f16 = np.float16
    cube = np.ascontiguousarray(np.asarray(cube, dtype=np.float32)).astype(f16)
    uu = np.asarray(uu, dtype=np.float32)
    vv = np.asarray(vv, dtype=np.float32)

    s1 = _apod1d()
    y = np.arange(NPIX, dtype=np.float64)

    # u-direction DFT constants (same for all cores)
    kj = np.arange(KU, dtype=np.float64) + ROW0
    ang_u = 2.0 * np.pi * np.outer(y, kj) / G
    cut = SC * np.cos(ang_u) / s1[:, None]
    sut = SC * np.sin(ang_u) / s1[:, None]

    def fold(a):
        return np.ascontiguousarray(
            a.reshape(8, P, KU).transpose(1, 0, 2)).astype(np.float16)

    cutf, sutf, nsutf = fold(cut), fold(sut), fold(-sut)

    tu, m0u, wu = _interp_host(uu)
    tv, m0v, wv = _interp_host(vv)
    rg = m0v - 2 - ROW0                # [0, 796)
    jg = m0u - 2 - ROW0                # [0, 796)
    assert rg.min() >= 0 and rg.max() < N_CORES * RPC
    assert jg.min() >= 0 and jg.max() + 7 < KU

    core_of = rg // RPC
    r = rg - core_of * RPC             # E row within core, [0, 100)
    # strip assignment: cols 396..408 may go to either strip (their two
    # residues differ by 4) -- balance buckets under the R_SLOTS cap
    CAP = R_SLOTS * P
    res0 = (r * SW + jg) % NRES
    sgrid = np.zeros(NVIS, np.int64)
    sgrid[jg > JHI] = 1
    for k in range(N_CORES):
        sel = np.where(core_of == k)[0]
        cnt = np.zeros((2, NRES), np.int64)
        flex = []
        for i in sel:
            if jg[i] < SOFF[1]:
                cnt[0, res0[i]] += 1
            elif jg[i] > JHI:
                cnt[1, (res0[i] - 4) % 8] += 1
            else:
                flex.append(i)
        for i in flex:
            b0, b1 = res0[i], (res0[i] - 4) % 8
            if cnt[0, b0] <= cnt[1, b1]:
                cnt[0, b0] += 1
                sgrid[i] = 0
            else:
                cnt[1, b1] += 1
                sgrid[i] = 1
        for i in flex:  # second pass: pull overfull buckets under the cap
            sg = sgrid[i]
            b0, b1 = res0[i], (res0[i] - 4) % 8
            bs, bo = (b0, b1) if sg == 0 else (b1, b0)
            if cnt[sg, bs] > CAP and cnt[1 - sg, bo] < CAP:
                cnt[sg, bs] -= 1
                cnt[1 - sg, bo] += 1
                sgrid[i] = 1 - sg
        assert cnt.max() <= CAP, f"core {k}: bucket {cnt.max()} > {CAP}"
    j0loc = jg - np.array(SOFF)[sgrid]
    assert j0loc.min() >= 0 and (j0loc + 7).max() < SW
    u = r * SW + j0loc
    res = u % NRES
    idx = u // NRES
    assert idx.max() < EVIEW_N

    w36 = (wu[:, :, None] * wv[:, None, :]) * WDESC   # [n, c, m] tap products

    in_maps = []
    meta = []
    for k in range(N_CORES):
        gidx = np.zeros((P, N_STREAMS * IDXCOLS), dtype=np.int16)
        w36k = np.zeros((P, V_SLOTS, 36), dtype=np.float32)
        meta_k = []
        for st in range(N_STREAMS):
            sg, rr = st // NRES, st % NRES
            order = np.where((core_of == k) & (sgrid == sg) & (res == rr))[0]
            n = len(order)
            assert n <= IDX_PER_S, f"core {k} stream {st} overflow: {n}"
            sl = np.arange(n)
            pp = sl % P
            vs = sl // P
            t = sl  # descriptor index == slot index
            block = np.zeros((16, IDXCOLS), dtype=np.int16)
            block[t % 16, t // 16] = idx[order].astype(np.int16)
            gidx[:, st * IDXCOLS:(st + 1) * IDXCOLS] = np.tile(block, (8, 1))
            w36k[pp, st * R_SLOTS + vs, :] = w36[order].reshape(n, 36)
            meta_k.append((order, pp, st * R_SLOTS + vs))
        # v-direction DFT constants for this core's slab rows
        kr = np.arange(R_ROWS, dtype=np.float64) + (ROW0 + RPC * k)
        ang_v = 2.0 * np.pi * np.outer(y, kr) / G
        blk = np.empty((NPIX, N1), dtype=np.float64)
        blk[:, :R_ROWS] = SC * np.cos(ang_v) / s1[:, None]
        blk[:, R_ROWS:] = -SC * np.sin(ang_v) / s1[:, None]
        cvt = np.ascontiguousarray(
            blk.reshape(8, P, N1).transpose(1, 0, 2)).astype(np.float16)

        in_maps.append({
            "cube": cube,
            "cvt": cvt,
            "cut": cutf,
            "sut": sutf,
            "nsut": nsutf,
            "gidx": gidx,
            "w36": w36k,
        })
        meta.append(meta_k)

    kv = vv * C1
    ku_ = uu * C1
    phase = np.exp(1j * (kv + ku_) * np.float32(NPIX / 2.0)).astype(np.complex64)
    return in_maps, meta, phase


def assemble(results, meta, phase):
    out = np.zeros((NCH, NVIS), dtype=np.complex64)
    for k in range(N_CORES):
        arr = results[k]["vis_out"].reshape(P, V_SLOTS, NCH, 2)
        for order, pp, rows in meta[k]:
            vals = arr[pp, rows]  # [n, NCH, 2]
            out[:, order] = (vals[..., 0] + 1j * vals[..., 1]).T
    return out * phase[None, :]


def kernel(cube, uu, vv):
    from concourse.bass_utils import run_bass_kernel_spmd

    nc = build_nc()
    in_maps, meta, phase = host_prep(cube, uu, vv)
    br = run_bass_kernel_spmd(
        nc, in_maps, list(range(N_CORES)),
        trace=bool(int(os.environ.get("NUFFT_TRACE", "0"))),
    )
    if br.exec_time_ns is not None:
        print(f"HW exec time: {br.exec_time_ns} ns")
    kernel.last_result = br
    return assemble(br.results, meta, phase)
